# revision 12
# baseline (speedup 1.0000x reference)
"""Causal self-attention (B=4, S=2048, C=1024, NH=16) on 8 TRN2 NeuronCores.

Sharding: core c = (batch b = c//2, head-group g = c%2 -> heads 8g..8g+7).
Per core: QKV projection (q/k produced transposed [hd, S] with head pairs
packed into 128 partitions; v natural [S, hd] plus a ones column for softmax
denominators), block-causal attention in [keys, q] orientation (scores
matmuls row-group-paired across the head pair, exp on ScalarE from PSUM,
PV matmul whose ones column accumulates denominators), 2-rank AllGather of
bf16 attention outputs within each batch pair, then an output-channel-split
final projection (core c outputs y[b, :, 512g:512g+512]) accumulated per
head-pair as each AllGather completes. Matmul inputs are stored bf16
(activations/weights); accumulation stays fp32 in PSUM.
"""
import sys

sys.path.insert(0, "/opt/trn_rl_repo")

import numpy as np

B, S, C = 4, 2048, 1024
NH, HD = 16, 64
NCORES = 8
HPG = 8          # heads per group (per core)
NPAIR = 4        # head pairs per core
NQC = S // 512   # q chunks of 512
MASK_VAL = -30000.0

_compiled = None


def _build():
    import concourse.bacc as bacc
    import concourse.mybir as mybir
    import concourse.tile as tile

    f32 = mybir.dt.float32
    bf16 = mybir.dt.bfloat16
    AF = mybir.ActivationFunctionType
    ADD = mybir.AluOpType.add
    MUL = mybir.AluOpType.mult

    nc = bacc.Bacc(None, target_bir_lowering=False, num_devices=NCORES)

    # ---- external parameters (per-core shards, host-prepared layouts) ----
    xT_ext = nc.declare_dram_parameter("xT", [C, S], bf16, isOutput=False)
    wqk_ext = nc.declare_dram_parameter("wqkT", [C, 1024], bf16, isOutput=False)
    qkb_ext = nc.declare_dram_parameter("qkb", [128, 8], f32, isOutput=False)
    wv_ext = nc.declare_dram_parameter("wvT", [C, 512], bf16, isOutput=False)
    vb_ext = nc.declare_dram_parameter("vb", [128, 512], f32, isOutput=False)
    ov_ext = nc.declare_dram_parameter("onesv", [128, 8], bf16, isOutput=False)
    msk_ext = nc.declare_dram_parameter("masks", [128, 2048], bf16, isOutput=False)
    id_ext = nc.declare_dram_parameter("ident", [128, 128], bf16, isOutput=False)
    wo_ext = nc.declare_dram_parameter("woT", [8, 128, 512], bf16, isOutput=False)
    wob_ext = nc.declare_dram_parameter("wob", [128, 512], f32, isOutput=False)
    y_ext = nc.declare_dram_parameter("y", [S, 512], f32, isOutput=True)

    # ---- internal DRAM ----
    ag_in = [[nc.dram_tensor(f"ag_in{p}_{h}", [128, S // 2], bf16)
              for h in range(2)] for p in range(NPAIR)]
    ag_out = [[nc.dram_tensor(f"ag_out{p}_{h}", [2, 128, S // 2], bf16)
               for h in range(2)] for p in range(NPAIR)]
    rd_dram = [nc.dram_tensor(f"rd{i}", [1, 512], f32) for i in range(32)]

    with tile.TileContext(nc) as tc:
        with (
            tc.tile_pool(name="persist", bufs=1) as pp,
            tc.tile_pool(name="attnp", bufs=1) as ap,
            tc.tile_pool(name="psum", bufs=1, space="PSUM") as psp,
        ):
            # ---- persistent tiles ----
            qkT = [pp.tile([128, S], bf16, tag=f"qkT{m}", name=f"qkT{m}")
                   for m in range(8)]
            vsb = [pp.tile([128, 520], bf16, tag=f"v{i}", name=f"v{i}")
                   for i in range(16)]
            qkb = pp.tile([128, 8], f32, tag="qkb")
            vb = pp.tile([128, 512], f32, tag="vb")
            onesv = pp.tile([128, 8], bf16, tag="onesv")
            msk = pp.tile([128, 2048], bf16, tag="msk")
            ident = pp.tile([128, 128], bf16, tag="ident")
            wo = [ap.tile([128, 512], bf16, tag=f"wo{k}", name=f"wo{k}")
                  for k in range(8)]
            wob = ap.tile([128, 512], f32, tag="wob")
            ysb = [ap.tile([128, 512], f32, tag=f"ysb{st}", name=f"ysb{st}")
                   for st in range(16)]
            def load_consts():
                nc.sync.dma_start(qkb[:], qkb_ext[:])
                nc.sync.dma_start(vb[:], vb_ext[:])
                nc.sync.dma_start(onesv[:], ov_ext[:])
                nc.sync.dma_start(msk[:], msk_ext[:])
                for k in range(8):
                    nc.sync.dma_start(wo[k][:], wo_ext[k][:])
                nc.sync.dma_start(wob[:], wob_ext[:])

            # ---------- attention inner bodies ----------
            def attn_chunk(hp, j):
                qt, kt = qkT[2 * hp], qkT[2 * hp + 1]
                a0, b0 = (2 * hp) * 65, (2 * hp + 1) * 65
                kmax = 4 * (j + 1)
                oTa = psp.tile([65, 512], f32, tag="oT", bufs=3, name="oTa")
                oTb = psp.tile([65, 512], f32, tag="oT", bufs=3, name="oTb")
                qsla = qt[0:64, j * 512:(j + 1) * 512]
                qslb = qt[64:128, j * 512:(j + 1) * 512]
                for kc in range(kmax):
                    d = kc - 4 * j
                    q0 = max(d, 0) * 128  # cols below q0 have no valid keys
                    diag = d >= 0
                    sAB = psp.tile([128, 1024], f32, tag="sAB", bufs=2, name="sAB")
                    nc.tensor.matmul(
                        sAB[:, q0:512],
                        kt[0:64, kc * 128:(kc + 1) * 128], qsla[:, q0:512],
                        start=True, stop=not diag, tile_position=(0, 0))
                    nc.tensor.matmul(
                        sAB[:, 512 + q0:1024],
                        kt[64:128, kc * 128:(kc + 1) * 128], qslb[:, q0:512],
                        start=True, stop=not diag, tile_position=(64, 0))
                    if diag:
                        m_sl = msk[:, d * 512 + q0:(d + 1) * 512]
                        nc.tensor.matmul(
                            sAB[:, q0:512], ident[:], m_sl,
                            start=False, stop=True)
                        nc.tensor.matmul(
                            sAB[:, 512 + q0:1024], ident[:], m_sl,
                            start=False, stop=True)
                    pAB = ap.tile([128, 1024], bf16, tag="pAB", bufs=4, name="pAB")
                    s_view = sAB[:].rearrange("p (h w) -> p h w", w=512)[:, :, q0:512]
                    p_view = pAB[:].rearrange("p (h w) -> p h w", w=512)[:, :, q0:512]
                    nc.scalar.activation(p_view, s_view, AF.Exp)
                    st_, sp_ = (kc == 0), (kc == kmax - 1)
                    nc.tensor.matmul(
                        oTa[:, q0:512], vsb[kc][:, a0:a0 + 65], pAB[:, q0:512],
                        start=st_, stop=sp_)
                    nc.tensor.matmul(
                        oTb[:, q0:512], vsb[kc][:, b0:b0 + 65],
                        pAB[:, 512 + q0:1024],
                        start=st_, stop=sp_)
                # normalize (1/denominator broadcast via DRAM bounce) + stage
                for head, oT in ((0, oTa), (1, oTb)):
                    dsb = ap.tile([65, 512], f32, tag="dsb", bufs=3, name="dsb")
                    nc.vector.tensor_copy(dsb[:], oT[:])
                    rdf = ap.tile([65, 512], f32, tag="rdf", bufs=3, name="rdf")
                    nc.vector.reciprocal_approx_fast(rdf[:], dsb[:])
                    scratch = rd_dram[(hp * NQC + j) * 2 + head]
                    nc.sync.dma_start(scratch[:], rdf[64:65, :])
                    bcs = ap.tile([64, 512], f32, tag="bcs", bufs=3, name="bcs")
                    nc.sync.dma_start(bcs[:], scratch[:].to_broadcast([64, 512]))
                    oTn = ap.tile([64, 512], bf16, tag="oTn", bufs=3, name="oTn")
                    nc.vector.tensor_tensor(oTn[:], oT[0:64, :], bcs[:], MUL)
                    nc.sync.dma_start(
                        ag_in[hp][j // 2][head * 64:(head + 1) * 64,
                                          (j % 2) * 512:(j % 2) * 512 + 512],
                        oTn[:])

            def gather(hp, half):
                nc.gpsimd.collective_compute(
                    "AllGather", mybir.AluOpType.bypass,
                    replica_groups=[[0, 1], [2, 3], [4, 5], [6, 7]],
                    ins=[ag_in[hp][half][:]],
                    outs=[ag_out[hp][half][:]],
                )

            def project(hp, half, final=False):
                for st in range(half * 8, half * 8 + 8):
                    ps = psp.tile([128, 512], f32, tag="acc", bufs=1, name="y_ps")
                    for r2 in range(2):
                        lt = ap.tile([128, 128], bf16, tag="lt", bufs=8, name="lt")
                        nc.sync.dma_start(
                            lt[:], ag_out[hp][half][r2][
                                :, (st % 8) * 128:(st % 8 + 1) * 128])
                        nc.tensor.matmul(
                            ps[:], lt[:], wo[hp * 2 + r2][:],
                            start=(r2 == 0), stop=(r2 == 1))
                    if hp == 0:
                        nc.vector.tensor_tensor(ysb[st][:], ps[:], wob[:], ADD)
                    else:
                        nc.vector.tensor_tensor(ysb[st][:], ps[:], ysb[st][:], ADD)
                    if final:
                        nc.sync.dma_start(
                            y_ext[st * 128:(st + 1) * 128, :], ysb[st][:])

            # ================= QKV projection interleaved with hp0 =========
            with tc.tile_pool(name="xt", bufs=1) as xp:
                xt = [xp.tile([128, S], bf16, tag=f"xt{k}", name=f"xt{k}")
                      for k in range(8)]

                with tc.tile_pool(name="wqv", bufs=1) as wqp:
                    wq = [wqp.tile([128, 1024], bf16, tag=f"wq{k}", name=f"wq{k}")
                          for k in range(8)]
                    wv = [wqp.tile([128, 512], bf16, tag=f"wv{k}", name=f"wv{k}")
                          for k in range(8)]

                    def qk_mjn(m, jn):
                        ps = psp.tile([128, 512], f32, tag="acc", bufs=1,
                                      name="qk_ps")
                        for k in range(8):
                            nc.tensor.matmul(
                                ps[:], wq[k][:, m * 128:(m + 1) * 128],
                                xt[k][:, jn * 512:(jn + 1) * 512],
                                start=(k == 0), stop=(k == 7))
                        nc.vector.tensor_scalar_add(
                            qkT[m][:, jn * 512:(jn + 1) * 512], ps[:],
                            qkb[:, m:m + 1])

                    def v_stile(st):
                        ps = psp.tile([128, 512], f32, tag="acc", bufs=1,
                                      name="v_ps")
                        for k in range(8):
                            nc.tensor.matmul(
                                ps[:], xt[k][:, st * 128:(st + 1) * 128],
                                wv[k][:],
                                start=(k == 0), stop=(k == 7))
                        nc.vector.tensor_tensor(
                            vsb[st][:].rearrange(
                                "p (h w) -> p h w", w=65)[:, :, 0:64],
                            ps[:].rearrange("p (h w) -> p h w", w=64),
                            vb[:].rearrange("p (h w) -> p h w", w=64), ADD)
                        nc.sync.dma_start(
                            vsb[st][:].rearrange(
                                "p (h w) -> p h w", w=65)[:, :, 64:65],
                            onesv[:].unsqueeze(2))

                    for k in range(8):
                        nc.sync.dma_start(
                            wq[k][:], wqk_ext[k * 128:(k + 1) * 128, :])
                    for jn in range(NQC):
                        for k in range(8):
                            nc.sync.dma_start(
                                xt[k][:, jn * 512:(jn + 1) * 512],
                                xT_ext[k * 128:(k + 1) * 128,
                                       jn * 512:(jn + 1) * 512])
                        if jn == 0:
                            nc.sync.dma_start(ident[:], id_ext[:])
                    for k in range(8):
                        nc.sync.dma_start(
                            wv[k][:], wv_ext[k * 128:(k + 1) * 128, :])
                    load_consts()

                    # jn-outer qk waves: compute follows each x column load
                    for jn in range(NQC):
                        for m in range(8):
                            qk_mjn(m, jn)
                    # v tiles provide PE filler for hp0 attention
                    for blk in range(4):
                        for st in range(4 * blk, 4 * blk + 4):
                            v_stile(st)
                        attn_chunk(0, blk)
                        if blk == 1:
                            gather(0, 0)
                    gather(0, 1)

            # ===== remaining head pairs: hp1/hp2 interleaved, hp3 + y-fill ====
            attn_chunk(1, 0)
            project(0, 0)
            attn_chunk(2, 0)
            project(0, 1)
            attn_chunk(1, 1)
            gather(1, 0)
            attn_chunk(2, 1)
            gather(2, 0)
            attn_chunk(1, 2)
            project(1, 0)
            attn_chunk(2, 2)
            project(2, 0)
            attn_chunk(1, 3)
            gather(1, 1)
            attn_chunk(2, 3)
            gather(2, 1)
            attn_chunk(3, 3)
            project(1, 1)
            attn_chunk(3, 2)
            gather(3, 1)
            attn_chunk(3, 1)
            project(2, 1)
            project(3, 1, final=True)
            attn_chunk(3, 0)
            gather(3, 0)
            project(3, 0, final=True)

    nc.compile()
    return nc


def _prep_inputs(x, Wqkv_w, Wqkv_b, Wo_w, Wo_b):
    """Build the 8 per-core input maps (host-side sharding / re-layout)."""
    import ml_dtypes

    bf = ml_dtypes.bfloat16
    x = np.asarray(x, dtype=np.float32)
    Wqkv_w = np.asarray(Wqkv_w, dtype=np.float32)
    Wqkv_b = np.asarray(Wqkv_b, dtype=np.float32)
    Wo_w = np.asarray(Wo_w, dtype=np.float32)
    Wo_b = np.asarray(Wo_b, dtype=np.float32)
    sc = 1.0 / np.sqrt(HD)

    pk = np.arange(128)[:, None]
    fq = np.arange(512)[None, :]
    masks = np.concatenate(
        [np.where(fq < pk + 128 * d, MASK_VAL, 0.0) for d in range(4)],
        axis=1).astype(bf)
    ident = np.eye(128, dtype=np.float32).astype(bf)
    onesv = np.ones([128, 8], np.float32).astype(bf)

    in_maps = []
    for c in range(NCORES):
        b, g = c // 2, c % 2
        heads = [8 * g + h for h in range(HPG)]

        blocks, bias_cols = [], []
        for p in range(NPAIR):
            ha, hb = heads[2 * p], heads[2 * p + 1]
            q_rows = np.concatenate(
                [Wqkv_w[ha * HD:(ha + 1) * HD], Wqkv_w[hb * HD:(hb + 1) * HD]]) * sc
            q_bias = np.concatenate(
                [Wqkv_b[ha * HD:(ha + 1) * HD], Wqkv_b[hb * HD:(hb + 1) * HD]]) * sc
            k_rows = np.concatenate(
                [Wqkv_w[C + ha * HD:C + (ha + 1) * HD],
                 Wqkv_w[C + hb * HD:C + (hb + 1) * HD]])
            k_bias = np.concatenate(
                [Wqkv_b[C + ha * HD:C + (ha + 1) * HD],
                 Wqkv_b[C + hb * HD:C + (hb + 1) * HD]])
            blocks += [q_rows, k_rows]
            bias_cols += [q_bias, k_bias]
        wqkT = np.concatenate([blk.T for blk in blocks], axis=1)  # [C, 1024]
        qkb = np.stack(bias_cols, axis=1)                         # [128, 8]

        wvT = np.zeros([C, 512], np.float32)
        vbv = np.zeros([512], np.float32)
        for hl, h in enumerate(heads):
            wvT[:, hl * 64:(hl + 1) * 64] = \
                Wqkv_w[2 * C + h * HD:2 * C + (h + 1) * HD].T
            vbv[hl * 64:(hl + 1) * 64] = Wqkv_b[2 * C + h * HD:2 * C + (h + 1) * HD]
        vb_rep = np.broadcast_to(vbv, (128, 512)).copy()

        woT = np.zeros([8, 128, 512], np.float32)
        for kc in range(8):
            hp, r2 = kc // 2, kc % 2
            ha, hb = 8 * r2 + 2 * hp, 8 * r2 + 2 * hp + 1
            cols = np.concatenate(
                [np.arange(ha * HD, (ha + 1) * HD),
                 np.arange(hb * HD, (hb + 1) * HD)])
            woT[kc] = Wo_w[g * 512:(g + 1) * 512, cols].T
        wob = np.broadcast_to(Wo_b[g * 512:(g + 1) * 512], (128, 512)).copy()

        in_maps.append({
            "xT": np.ascontiguousarray(x[b].T).astype(bf),
            "wqkT": np.ascontiguousarray(wqkT).astype(bf),
            "qkb": np.ascontiguousarray(qkb),
            "wvT": wvT.astype(bf),
            "vb": vb_rep,
            "onesv": onesv,
            "masks": masks,
            "ident": ident,
            "woT": woT.astype(bf),
            "wob": wob,
        })
    return in_maps


def run(inputs, trace=False):
    """Compile (cached), shard, run on 8 cores, gather. Returns (y, result)."""
    global _compiled
    from concourse.bass_utils import run_bass_kernel_spmd

    if _compiled is None:
        _compiled = _build()
    in_maps = _prep_inputs(
        inputs["x"], inputs["Wqkv_w"], inputs["Wqkv_b"],
        inputs["Wo_w"], inputs["Wo_b"])
    res = run_bass_kernel_spmd(
        _compiled, in_maps, list(range(NCORES)), trace=trace)
    y = np.empty([B, S, C], np.float32)
    for c in range(NCORES):
        b, g = c // 2, c % 2
        y[b, :, g * 512:(g + 1) * 512] = res.results[c]["y"]
    return y, res


def kernel(**inputs) -> np.ndarray:
    y, _ = run(inputs, trace=False)
    return y


# revision 14
# speedup vs baseline: 1.0455x; 1.0455x over previous
"""Causal self-attention (B=4, S=2048, C=1024, NH=16) on 8 TRN2 NeuronCores.

Sharding: core c = (batch b = c//2, head-group g = c%2 -> heads 8g..8g+7).
Per core: QKV projection (q/k produced transposed [hd, S] with head pairs
packed into 128 partitions; v natural [S, hd] plus a ones column for softmax
denominators), block-causal attention in [keys, q] orientation (scores
matmuls row-group-paired across the head pair, exp on ScalarE from PSUM,
PV matmul whose ones column accumulates denominators), 2-rank AllGather of
bf16 attention outputs within each batch pair, then an output-channel-split
final projection (core c outputs y[b, :, 512g:512g+512]) accumulated per
head-pair as each AllGather completes. Matmul inputs are stored bf16
(activations/weights); accumulation stays fp32 in PSUM.
"""
import sys

sys.path.insert(0, "/opt/trn_rl_repo")

import numpy as np

B, S, C = 4, 2048, 1024
NH, HD = 16, 64
NCORES = 8
HPG = 8          # heads per group (per core)
NPAIR = 4        # head pairs per core
NQC = S // 512   # q chunks of 512
MASK_VAL = -30000.0

_compiled = None


def _build():
    import concourse.bacc as bacc
    import concourse.mybir as mybir
    import concourse.tile as tile

    f32 = mybir.dt.float32
    bf16 = mybir.dt.bfloat16
    AF = mybir.ActivationFunctionType
    ADD = mybir.AluOpType.add
    MUL = mybir.AluOpType.mult

    nc = bacc.Bacc(None, target_bir_lowering=False, num_devices=NCORES)

    # ---- external parameters (per-core shards, host-prepared layouts) ----
    xT_ext = nc.declare_dram_parameter("xT", [C, S], bf16, isOutput=False)
    wqk_ext = nc.declare_dram_parameter("wqkT", [C, 1024], bf16, isOutput=False)
    qkb_ext = nc.declare_dram_parameter("qkb", [128, 8], f32, isOutput=False)
    wv_ext = nc.declare_dram_parameter("wvT", [C, 512], bf16, isOutput=False)
    vb_ext = nc.declare_dram_parameter("vb", [128, 512], f32, isOutput=False)
    ov_ext = nc.declare_dram_parameter("onesv", [128, 8], bf16, isOutput=False)
    msk_ext = nc.declare_dram_parameter("masks", [128, 2048], bf16, isOutput=False)
    id_ext = nc.declare_dram_parameter("ident", [128, 128], bf16, isOutput=False)
    wo_ext = nc.declare_dram_parameter("woT", [8, 128, 512], bf16, isOutput=False)
    wob_ext = nc.declare_dram_parameter("wob", [128, 512], f32, isOutput=False)
    y_ext = nc.declare_dram_parameter("y", [S, 512], f32, isOutput=True)

    # ---- internal DRAM ----
    ag_in = [[nc.dram_tensor(f"ag_in{p}_{h}", [128, S // 2], bf16)
              for h in range(2)] for p in range(NPAIR)]
    ag_out = [[nc.dram_tensor(f"ag_out{p}_{h}", [2, 128, S // 2], bf16)
               for h in range(2)] for p in range(NPAIR)]
    rd_dram = [nc.dram_tensor(f"rd{i}", [1, 512], f32) for i in range(32)]

    with tile.TileContext(nc) as tc:
        with (
            tc.tile_pool(name="persist", bufs=1) as pp,
            tc.tile_pool(name="attnp", bufs=1) as ap,
            tc.tile_pool(name="psum", bufs=1, space="PSUM") as psp,
        ):
            # ---- persistent tiles ----
            qkT = [pp.tile([128, S], bf16, tag=f"qkT{m}", name=f"qkT{m}")
                   for m in range(8)]
            vsb = [pp.tile([128, 520], bf16, tag=f"v{i}", name=f"v{i}")
                   for i in range(16)]
            qkb = pp.tile([128, 8], f32, tag="qkb")
            vb = pp.tile([128, 512], f32, tag="vb")
            onesv = pp.tile([128, 8], bf16, tag="onesv")
            msk = pp.tile([128, 2048], bf16, tag="msk")
            ident = pp.tile([128, 128], bf16, tag="ident")
            wo = [ap.tile([128, 512], bf16, tag=f"wo{k}", name=f"wo{k}")
                  for k in range(8)]
            wob = ap.tile([128, 512], f32, tag="wob")
            ysb = [ap.tile([128, 512], f32, tag=f"ysb{st}", name=f"ysb{st}")
                   for st in range(16)]
            def load_consts():
                nc.sync.dma_start(qkb[:], qkb_ext[:])
                nc.sync.dma_start(vb[:], vb_ext[:])
                nc.sync.dma_start(onesv[:], ov_ext[:])
                nc.sync.dma_start(msk[:], msk_ext[:])
                for k in range(8):
                    nc.sync.dma_start(wo[k][:], wo_ext[k][:])
                nc.sync.dma_start(wob[:], wob_ext[:])

            # ---------- attention inner bodies ----------
            def attn_chunk(hp, j):
                qt, kt = qkT[2 * hp], qkT[2 * hp + 1]
                a0, b0 = (2 * hp) * 65, (2 * hp + 1) * 65
                kmax = 4 * (j + 1)
                oTa = psp.tile([65, 512], f32, tag="oT", bufs=3, name="oTa")
                oTb = psp.tile([65, 512], f32, tag="oT", bufs=3, name="oTb")
                qsla = qt[0:64, j * 512:(j + 1) * 512]
                qslb = qt[64:128, j * 512:(j + 1) * 512]
                for kc in range(kmax):
                    d = kc - 4 * j
                    q0 = max(d, 0) * 128  # cols below q0 have no valid keys
                    diag = d >= 0
                    sAB = psp.tile([128, 1024], f32, tag="sAB", bufs=2, name="sAB")
                    nc.tensor.matmul(
                        sAB[:, q0:512],
                        kt[0:64, kc * 128:(kc + 1) * 128], qsla[:, q0:512],
                        start=True, stop=not diag, tile_position=(0, 0))
                    nc.tensor.matmul(
                        sAB[:, 512 + q0:1024],
                        kt[64:128, kc * 128:(kc + 1) * 128], qslb[:, q0:512],
                        start=True, stop=not diag, tile_position=(64, 0))
                    if diag:
                        m_sl = msk[:, d * 512 + q0:(d + 1) * 512]
                        nc.tensor.matmul(
                            sAB[:, q0:512], ident[:], m_sl,
                            start=False, stop=True)
                        nc.tensor.matmul(
                            sAB[:, 512 + q0:1024], ident[:], m_sl,
                            start=False, stop=True)
                    pAB = ap.tile([128, 1024], bf16, tag="pAB", bufs=4, name="pAB")
                    s_view = sAB[:].rearrange("p (h w) -> p h w", w=512)[:, :, q0:512]
                    p_view = pAB[:].rearrange("p (h w) -> p h w", w=512)[:, :, q0:512]
                    nc.scalar.activation(p_view, s_view, AF.Exp)
                    st_, sp_ = (kc == 0), (kc == kmax - 1)
                    nc.tensor.matmul(
                        oTa[:, q0:512], vsb[kc][:, a0:a0 + 65], pAB[:, q0:512],
                        start=st_, stop=sp_)
                    nc.tensor.matmul(
                        oTb[:, q0:512], vsb[kc][:, b0:b0 + 65],
                        pAB[:, 512 + q0:1024],
                        start=st_, stop=sp_)
                # normalize (1/denominator broadcast via DRAM bounce) + stage
                for head, oT in ((0, oTa), (1, oTb)):
                    dsb = ap.tile([65, 512], f32, tag="dsb", bufs=3, name="dsb")
                    nc.vector.tensor_copy(dsb[:], oT[:])
                    rdf = ap.tile([65, 512], f32, tag="rdf", bufs=3, name="rdf")
                    nc.vector.reciprocal_approx_fast(rdf[:], dsb[:])
                    scratch = rd_dram[(hp * NQC + j) * 2 + head]
                    nc.sync.dma_start(scratch[:], rdf[64:65, :])
                    bcs = ap.tile([64, 512], f32, tag="bcs", bufs=3, name="bcs")
                    nc.sync.dma_start(bcs[:], scratch[:].to_broadcast([64, 512]))
                    oTn = ap.tile([64, 512], bf16, tag="oTn", bufs=3, name="oTn")
                    nc.vector.tensor_tensor(oTn[:], oT[0:64, :], bcs[:], MUL)
                    nc.sync.dma_start(
                        ag_in[hp][j // 2][head * 64:(head + 1) * 64,
                                          (j % 2) * 512:(j % 2) * 512 + 512],
                        oTn[:])

            def gather(hp, half):
                nc.gpsimd.collective_compute(
                    "AllGather", mybir.AluOpType.bypass,
                    replica_groups=[[0, 1], [2, 3], [4, 5], [6, 7]],
                    ins=[ag_in[hp][half][:]],
                    outs=[ag_out[hp][half][:]],
                )

            def project(hp, half, final=False):
                for st in range(half * 8, half * 8 + 8):
                    ps = psp.tile([128, 512], f32, tag="acc", bufs=1, name="y_ps")
                    for r2 in range(2):
                        lt = ap.tile([128, 128], bf16, tag="lt", bufs=8, name="lt")
                        nc.sync.dma_start(
                            lt[:], ag_out[hp][half][r2][
                                :, (st % 8) * 128:(st % 8 + 1) * 128])
                        nc.tensor.matmul(
                            ps[:], lt[:], wo[hp * 2 + r2][:],
                            start=(r2 == 0), stop=(r2 == 1))
                    if hp == 0:
                        nc.vector.tensor_tensor(ysb[st][:], ps[:], wob[:], ADD)
                    else:
                        nc.vector.tensor_tensor(ysb[st][:], ps[:], ysb[st][:], ADD)
                    if final:
                        nc.sync.dma_start(
                            y_ext[st * 128:(st + 1) * 128, :], ysb[st][:])

            # ================= QKV projection interleaved with hp0 =========
            with tc.tile_pool(name="xt", bufs=1) as xp:
                xt = [xp.tile([128, S], bf16, tag=f"xt{k}", name=f"xt{k}")
                      for k in range(8)]

                with tc.tile_pool(name="wqv", bufs=1) as wqp:
                    wq = [wqp.tile([128, 1024], bf16, tag=f"wq{k}", name=f"wq{k}")
                          for k in range(8)]
                    wv = [wqp.tile([128, 512], bf16, tag=f"wv{k}", name=f"wv{k}")
                          for k in range(8)]

                    def qk_mjn(m, jn):
                        ps = psp.tile([128, 512], f32, tag="acc", bufs=1,
                                      name="qk_ps")
                        for k in range(8):
                            nc.tensor.matmul(
                                ps[:], wq[k][:, m * 128:(m + 1) * 128],
                                xt[k][:, jn * 512:(jn + 1) * 512],
                                start=(k == 0), stop=(k == 7))
                        nc.vector.tensor_scalar_add(
                            qkT[m][:, jn * 512:(jn + 1) * 512], ps[:],
                            qkb[:, m:m + 1])

                    def v_stile(st):
                        ps = psp.tile([128, 512], f32, tag="acc", bufs=1,
                                      name="v_ps")
                        for k in range(8):
                            nc.tensor.matmul(
                                ps[:], xt[k][:, st * 128:(st + 1) * 128],
                                wv[k][:],
                                start=(k == 0), stop=(k == 7))
                        nc.vector.tensor_tensor(
                            vsb[st][:].rearrange(
                                "p (h w) -> p h w", w=65)[:, :, 0:64],
                            ps[:].rearrange("p (h w) -> p h w", w=64),
                            vb[:].rearrange("p (h w) -> p h w", w=64), ADD)
                        nc.sync.dma_start(
                            vsb[st][:].rearrange(
                                "p (h w) -> p h w", w=65)[:, :, 64:65],
                            onesv[:].unsqueeze(2))

                    for k in range(8):
                        nc.sync.dma_start(
                            wq[k][:], wqk_ext[k * 128:(k + 1) * 128, :])
                    for jn in range(NQC):
                        for k in range(8):
                            nc.sync.dma_start(
                                xt[k][:, jn * 512:(jn + 1) * 512],
                                xT_ext[k * 128:(k + 1) * 128,
                                       jn * 512:(jn + 1) * 512])
                        if jn == 0:
                            nc.sync.dma_start(ident[:], id_ext[:])
                    for k in range(8):
                        nc.sync.dma_start(
                            wv[k][:], wv_ext[k * 128:(k + 1) * 128, :])
                    load_consts()

                    # interleave: qk m-tiles / v s-tiles feed hp0 attention
                    for blk in range(4):
                        qk_mjn(2 * blk, 0)
                        qk_mjn(2 * blk, 1)
                        qk_mjn(2 * blk, 2)
                        qk_mjn(2 * blk, 3)
                        qk_mjn(2 * blk + 1, 0)
                        qk_mjn(2 * blk + 1, 1)
                        qk_mjn(2 * blk + 1, 2)
                        qk_mjn(2 * blk + 1, 3)
                        for st in range(4 * blk, 4 * blk + 4):
                            v_stile(st)
                        attn_chunk(0, blk)
                        if blk == 1:
                            gather(0, 0)
                    gather(0, 1)

            # ===== remaining head pairs: hp1/hp2 interleaved, hp3 + y-fill ====
            attn_chunk(1, 0)
            project(0, 0)
            attn_chunk(2, 0)
            project(0, 1)
            attn_chunk(1, 1)
            gather(1, 0)
            attn_chunk(2, 1)
            gather(2, 0)
            attn_chunk(1, 2)
            project(1, 0)
            attn_chunk(2, 2)
            project(2, 0)
            attn_chunk(1, 3)
            gather(1, 1)
            attn_chunk(2, 3)
            gather(2, 1)
            attn_chunk(3, 3)
            project(1, 1)
            attn_chunk(3, 2)
            gather(3, 1)
            attn_chunk(3, 1)
            project(2, 1)
            project(3, 1, final=True)
            attn_chunk(3, 0)
            gather(3, 0)
            project(3, 0, final=True)

    nc.compile()
    return nc


def _prep_inputs(x, Wqkv_w, Wqkv_b, Wo_w, Wo_b):
    """Build the 8 per-core input maps (host-side sharding / re-layout)."""
    import ml_dtypes

    bf = ml_dtypes.bfloat16
    x = np.asarray(x, dtype=np.float32)
    Wqkv_w = np.asarray(Wqkv_w, dtype=np.float32)
    Wqkv_b = np.asarray(Wqkv_b, dtype=np.float32)
    Wo_w = np.asarray(Wo_w, dtype=np.float32)
    Wo_b = np.asarray(Wo_b, dtype=np.float32)
    sc = 1.0 / np.sqrt(HD)

    pk = np.arange(128)[:, None]
    fq = np.arange(512)[None, :]
    masks = np.concatenate(
        [np.where(fq < pk + 128 * d, MASK_VAL, 0.0) for d in range(4)],
        axis=1).astype(bf)
    ident = np.eye(128, dtype=np.float32).astype(bf)
    onesv = np.ones([128, 8], np.float32).astype(bf)

    in_maps = []
    for c in range(NCORES):
        b, g = c // 2, c % 2
        heads = [8 * g + h for h in range(HPG)]

        blocks, bias_cols = [], []
        for p in range(NPAIR):
            ha, hb = heads[2 * p], heads[2 * p + 1]
            q_rows = np.concatenate(
                [Wqkv_w[ha * HD:(ha + 1) * HD], Wqkv_w[hb * HD:(hb + 1) * HD]]) * sc
            q_bias = np.concatenate(
                [Wqkv_b[ha * HD:(ha + 1) * HD], Wqkv_b[hb * HD:(hb + 1) * HD]]) * sc
            k_rows = np.concatenate(
                [Wqkv_w[C + ha * HD:C + (ha + 1) * HD],
                 Wqkv_w[C + hb * HD:C + (hb + 1) * HD]])
            k_bias = np.concatenate(
                [Wqkv_b[C + ha * HD:C + (ha + 1) * HD],
                 Wqkv_b[C + hb * HD:C + (hb + 1) * HD]])
            blocks += [q_rows, k_rows]
            bias_cols += [q_bias, k_bias]
        wqkT = np.concatenate([blk.T for blk in blocks], axis=1)  # [C, 1024]
        qkb = np.stack(bias_cols, axis=1)                         # [128, 8]

        wvT = np.zeros([C, 512], np.float32)
        vbv = np.zeros([512], np.float32)
        for hl, h in enumerate(heads):
            wvT[:, hl * 64:(hl + 1) * 64] = \
                Wqkv_w[2 * C + h * HD:2 * C + (h + 1) * HD].T
            vbv[hl * 64:(hl + 1) * 64] = Wqkv_b[2 * C + h * HD:2 * C + (h + 1) * HD]
        vb_rep = np.broadcast_to(vbv, (128, 512)).copy()

        woT = np.zeros([8, 128, 512], np.float32)
        for kc in range(8):
            hp, r2 = kc // 2, kc % 2
            ha, hb = 8 * r2 + 2 * hp, 8 * r2 + 2 * hp + 1
            cols = np.concatenate(
                [np.arange(ha * HD, (ha + 1) * HD),
                 np.arange(hb * HD, (hb + 1) * HD)])
            woT[kc] = Wo_w[g * 512:(g + 1) * 512, cols].T
        wob = np.broadcast_to(Wo_b[g * 512:(g + 1) * 512], (128, 512)).copy()

        in_maps.append({
            "xT": np.ascontiguousarray(x[b].T).astype(bf),
            "wqkT": np.ascontiguousarray(wqkT).astype(bf),
            "qkb": np.ascontiguousarray(qkb),
            "wvT": wvT.astype(bf),
            "vb": vb_rep,
            "onesv": onesv,
            "masks": masks,
            "ident": ident,
            "woT": woT.astype(bf),
            "wob": wob,
        })
    return in_maps


def run(inputs, trace=False):
    """Compile (cached), shard, run on 8 cores, gather. Returns (y, result)."""
    global _compiled
    from concourse.bass_utils import run_bass_kernel_spmd

    if _compiled is None:
        _compiled = _build()
    in_maps = _prep_inputs(
        inputs["x"], inputs["Wqkv_w"], inputs["Wqkv_b"],
        inputs["Wo_w"], inputs["Wo_b"])
    res = run_bass_kernel_spmd(
        _compiled, in_maps, list(range(NCORES)), trace=trace)
    y = np.empty([B, S, C], np.float32)
    for c in range(NCORES):
        b, g = c // 2, c % 2
        y[b, :, g * 512:(g + 1) * 512] = res.results[c]["y"]
    return y, res


def kernel(**inputs) -> np.ndarray:
    y, _ = run(inputs, trace=False)
    return y
